# revision 16
# baseline (speedup 1.0000x reference)
"""CfC cell (dense MLP) Trainium2 Bass kernel.

Reference math (fp32):
    x  = concat([input, hx], axis=1)                  # [B, 768]
    h  = 1.7159 * tanh(0.666 * (x @ Wb.T + bb))       # [B, 1024]
    ff1 = tanh(h @ W1.T + b1)                         # [B, 512]
    ff2 = tanh(h @ W2.T + b2)
    t_a = h @ Wa.T + ba
    t_b = h @ Wt.T + bt
    t   = sigmoid(t_a * ts + t_b)
    out = ff1 * (1 - t) + t * ff2

Strategy: data-parallel over batch across 8 NeuronCores (2048 rows each).
Device layouts (contraction dim on partitions, fp16 matmul operands):
  - xT   [768, 2048]        x transposed
  - WbT  [768, 1024]        layer-1 stationary
  - WHF  [2, 1024, 512]     1.7159*W{1,2}.T, ff-head moving operands (fp16)
  - WH8  [2, 4, 128, 2, 512] K8*1.7159*W{a,t}.T as fp8e4m3 DoubleRow pairs
  - BBP  [128, 8]           0.666*bb (ACT bias for layer-1 tanh)
  - BHF/BH8 [2, 128, 512]   head biases broadcast across partitions (BH8 scaled K8)
  - TSP  [128, 16]          ts, column mi = batch subtile mi

Layer 1 emits hT [units, batch] twice from PSUM: fp16 tiles (ff heads) and
fp8 pair tiles (t heads).  The t-gate heads run as fp8 DoubleRow matmuls
(~1.44x PE rate); sigmoid's <=0.25 slope keeps the fp8 noise well inside the
accuracy budget.  All head biases are preloaded into PSUM by the (otherwise
idle) Pool engine so neither DVE nor ACT spends time on bias adds; the
accumulation groups then run start=False.  The final interpolation runs in
fp16 on DVE (2x rate).
"""

import os
import sys

import numpy as np

if "/opt/trn_rl_repo" not in sys.path:
    sys.path.insert(0, "/opt/trn_rl_repo")

B, IN, HID, UNITS = 16384, 256, 512, 1024
CAT = IN + HID  # 768
N_CORES = 8
BS = B // N_CORES  # 2048 per core
P = 128
NK1 = CAT // P    # 6 contraction tiles, layer 1
NU = UNITS // P   # 8 unit tiles
NJ = NU // 2      # 4 unit pair tiles (DoubleRow)
K8 = 1024.0       # fp8 weight pre-scale (sigmoid applies 1/K8)

_cache = {}


def build_nc(bs=BS, chunk=512):
    """Build the single-core Bass program (same program runs SPMD on 8 cores)."""
    from concourse import bacc, tile, mybir

    AF = mybir.ActivationFunctionType
    ALU = mybir.AluOpType
    PM = mybir.MatmulPerfMode
    F32 = mybir.dt.float32
    F16 = mybir.dt.float16
    F8 = mybir.dt.float8e4

    nchunk = bs // chunk
    nm = chunk // P  # batch subtiles per chunk
    NM = bs // P     # total batch subtiles

    nc = bacc.Bacc("TRN2", target_bir_lowering=False, debug=False,
                   num_devices=N_CORES)

    # xt/wbt are pre-tiled host-side so every startup DMA is one fully
    # contiguous block (the PE outruns scattered 1KB-row transfers)
    xt_d = nc.dram_tensor("xt", [nchunk, NK1, P, chunk], F16,
                          kind="ExternalInput").ap()
    wbt_d = nc.dram_tensor("wbt", [NK1, 2, P, UNITS // 2], F16,
                           kind="ExternalInput").ap()
    whf_d = nc.dram_tensor("whf", [2, UNITS, HID], F16, kind="ExternalInput").ap()
    wh8_d = nc.dram_tensor("wh8", [2, NJ, P, 2, HID], F8, kind="ExternalInput").ap()
    bbp_d = nc.dram_tensor("bbp", [P, NU], F32, kind="ExternalInput").ap()
    bhf_d = nc.dram_tensor("bhf", [2, P, HID], F32, kind="ExternalInput").ap()
    bh8_d = nc.dram_tensor("bh8", [2, P, HID], F32, kind="ExternalInput").ap()
    tsp_d = nc.dram_tensor("tsp", [P, NM], F32, kind="ExternalInput").ap()
    out_d = nc.dram_tensor("out", [bs, HID], F32, kind="ExternalOutput").ap()

    with tile.TileContext(nc) as tc:
        with (
            tc.tile_pool(name="const", bufs=1) as const,
            tc.tile_pool(name="xp", bufs=4) as xp,
            tc.tile_pool(name="hp", bufs=1) as hp,
            tc.tile_pool(name="tp", bufs=2) as tp,
            tc.tile_pool(name="op", bufs=3) as op,
            tc.tile_pool(name="psp", bufs=8, space="PSUM") as psp,
        ):
            # --- PE warmup: ramp the p-state while startup DMAs stream ----
            warm = const.tile([P, 512], F16, tag="warm")
            nc.vector.memset(warm[:], 0.0)
            for _ in range(3):
                wps = psp.tile([P, 512], F32, tag="ps")
                nc.tensor.matmul(wps[:], warm[:, 0:P], warm[:],
                                 start=True, stop=True)

            def load_x(bc):
                xts = []
                for c in range(NK1):
                    t = xp.tile([P, chunk], F16, tag=f"x{c}")
                    (nc.sync if c % 2 == 0 else nc.scalar).dma_start(
                        t[:], xt_d[bc, c])
                    xts.append(t)
                return xts

            # first-chunk x tiles on the Sync DGE queue, layer-1 weights on
            # the Scalar DGE queue (idle until layer-1 ACT ~18us in): both
            # issue streams run in parallel so chunk 0 is fully resident
            # ~3.5us sooner than a single serialized queue
            HALF = UNITS // 2
            wb_sb = [[None, None] for _ in range(NK1)]
            xts0 = []
            for c in range(NK1):
                t = xp.tile([P, chunk], F16, tag=f"x{c}")
                # alternate the big x transfers across both queues so the
                # c-outer accumulation group never outruns the DMAs
                (nc.sync if c % 2 == 0 else nc.scalar).dma_start(
                    t[:], xt_d[0, c])
                xts0.append(t)
                t = const.tile([P, HALF], F16, tag=f"wbh{c}_0")
                (nc.scalar if c % 2 == 0 else nc.sync).dma_start(
                    t[:], wbt_d[c, 0])
                wb_sb[c][0] = t

            # small constants early (bb gates every layer-1 activation)
            bb_sb = const.tile([P, NU], F32, tag="bb")
            nc.scalar.dma_start(bb_sb[:], bbp_d[:])

            for c in range(NK1):
                t = const.tile([P, HALF], F16, tag=f"wbh{c}_1")
                (nc.scalar if c % 2 == 0 else nc.sync).dma_start(
                    t[:], wbt_d[c, 1])
                wb_sb[c][1] = t

            # all remaining x chunks next: layer-1 for every chunk runs
            # before any layer-2, so the head weights are needed only ~45us in
            xts_all = [xts0] + [load_x(bc) for bc in range(1, nchunk)]

            # t-head fp8 weights (needed first in each layer-2 m-tile)
            wh8_sb = [[None] * NJ for _ in range(2)]
            for k in range(2):
                for j in range(NJ):
                    t = const.tile([P, 2, HID], F8, tag=f"wh8_{k}_{j}")
                    nc.sync.dma_start(t[:], wh8_d[k, j])
                    wh8_sb[k][j] = t

            # ff-head fp16 weights
            whf_sb = [[None] * NU for _ in range(2)]
            for k in range(2):
                for u in range(NU):
                    t = const.tile([P, HID], F16, tag=f"whf_{k}_{u}")
                    nc.sync.dma_start(t[:], whf_d[k, u * P:(u + 1) * P, :])
                    whf_sb[k][u] = t

            # biases + ts (gate the first layer-2 psum preloads / stt)
            bh8_sb = [None, None]
            bhf_sb = [None, None]
            for k in range(2):
                t = const.tile([P, HID], F32, tag=f"bh8_{k}", name=f"bh8_{k}")
                nc.sync.dma_start(t[:], bh8_d[k])
                bh8_sb[k] = t
            for k in range(2):
                t = const.tile([P, HID], F32, tag=f"bhf_{k}", name=f"bhf_{k}")
                nc.sync.dma_start(t[:], bhf_d[k])
                bhf_sb[k] = t
            ts_sb = const.tile([P, NM], F32, tag="ts")
            nc.sync.dma_start(ts_sb[:], tsp_d[:])

            # persistent h storage: fp16 per u-tile, fp8 pairs per j-tile
            h16 = [[None] * NU for _ in range(nchunk)]
            h8 = [[None] * NJ for _ in range(nchunk)]
            for bc in range(nchunk):
                for u in range(NU):
                    h16[bc][u] = hp.tile([P, chunk], F16, tag=f"h16_{bc}_{u}",
                                         name=f"h16_{bc}_{u}")
                for j in range(NJ):
                    h8[bc][j] = hp.tile([P, 2, chunk], F8, tag=f"h8_{bc}_{j}",
                                        name=f"h8_{bc}_{j}")

            def layer1(bc):
                """hT[u] = tanh(0.666*(WbT.T @ xT) + 0.666*bb), fp16 + fp8 out.

                c-outer accumulation in two u-half-groups: the first matmul
                only needs xts[0] + wb half, so PE starts as soon as the
                first ~0.26 MB of DMA lands.
                """
                xts = xts_all[bc]
                for half in range(2):
                    pss = [psp.tile([P, chunk], F32, tag="ps", name=f"psl1_{j}")
                           for j in range(NU // 2)]
                    for c in range(NK1):
                        for j in range(NU // 2):
                            nc.tensor.matmul(
                                pss[j][:],
                                wb_sb[c][half][:, j * P:(j + 1) * P],
                                xts[c][:],
                                start=(c == 0), stop=(c == NK1 - 1))
                    for j in range(NU // 2):
                        u = half * (NU // 2) + j
                        nc.scalar.activation(h16[bc][u][:], pss[j][:], AF.Tanh,
                                             bias=bb_sb[:, u:u + 1], scale=0.666)
                        nc.scalar.activation(h8[bc][u // 2][:, u % 2, :],
                                             pss[j][:], AF.Tanh,
                                             bias=bb_sb[:, u:u + 1], scale=0.666)

            def layer2(bc):
                hts = h16[bc]
                h8s = h8[bc]
                for m in range(nm):
                    mi = bc * nm + m
                    last = (bc == nchunk - 1) and (m == nm - 1)
                    mc = slice(m * P, (m + 1) * P)

                    # t-path heads (fp8 DoubleRow), biases preloaded by Pool
                    def mm_t(k):
                        ps = psp.tile([P, HID], F32, tag="ps")
                        nc.vector.tensor_copy(ps[:], bh8_sb[k][:])
                        for j in range(NJ):
                            nc.tensor.matmul(
                                ps[:],
                                h8s[j][:, :, mc],
                                wh8_sb[k][j][:],
                                start=False, stop=(j == NJ - 1),
                                perf_mode=PM.DoubleRow,
                                skip_group_check=True)
                        return ps

                    def mm_ff(k):
                        ps = psp.tile([P, HID], F32, tag="ps")
                        nc.vector.tensor_copy(ps[:], bhf_sb[k][:])
                        for u in range(NU):
                            nc.tensor.matmul(
                                ps[:],
                                hts[u][:, mc],
                                whf_sb[k][u][:],
                                start=False, stop=(u == NU - 1),
                                skip_group_check=True)
                        return ps

                    # t-path first so the sigmoid chain overlaps the ff matmuls.
                    # DVE may read only one PSUM operand per op: stage pb in
                    # SBUF via ACT (overlaps pa's matmuls).
                    pb = mm_t(1)
                    ub = tp.tile([P, HID], F32, tag="ub")
                    nc.scalar.copy(ub[:], pb[:])
                    pa = mm_t(0)
                    w = tp.tile([P, HID], F32, tag="w")
                    nc.vector.scalar_tensor_tensor(
                        w[:], pa[:], ts_sb[:, mi:mi + 1], ub[:],
                        op0=ALU.mult, op1=ALU.add)
                    tt = tp.tile([P, HID], F16, tag="tt")
                    nc.scalar.activation(tt[:], w[:], AF.Sigmoid, scale=1.0 / K8)

                    p1 = mm_ff(0)
                    f1 = tp.tile([P, HID], F16, tag="f1")
                    nc.scalar.activation(f1[:], p1[:], AF.Tanh)

                    f2 = tp.tile([P, HID], F16, tag="f2")
                    o = op.tile([P, HID], F32, tag="o")

                    def combine(cs, qi, p2t, lo):
                        """o[:, cs] = f1 + tt*(f2 - f1); p2t covers cols lo:."""
                        ls = slice(cs.start - lo, cs.stop - lo)
                        d = tp.tile([P, HID], F16, tag="d")
                        nc.scalar.activation(f2[:, cs], p2t[:, ls], AF.Tanh)
                        nc.vector.tensor_sub(d[:, cs], f2[:, cs], f1[:, cs])
                        nc.vector.tensor_mul(d[:, cs], d[:, cs], tt[:, cs])
                        nc.vector.tensor_add(o[:, cs], d[:, cs], f1[:, cs])
                        # alternate DGE queues so the tail quarters' DMA
                        # issues (~600ns each) overlap instead of serializing
                        eng = nc.scalar if (last and qi % 2) else nc.sync
                        eng.dma_start(out_d[mi * P:(mi + 1) * P, cs], o[:, cs])

                    if not last:
                        p2 = mm_ff(1)
                        combine(slice(0, HID), 0, p2, 0)
                    else:
                        # last tile: ff2 in two 256-col half-groups so the
                        # first half's chain overlaps the second half's
                        # matmuls, and quartered chains pipeline ACT/DVE/DMA
                        H2 = HID // 2
                        for hi in range(2):
                            hs = slice(hi * H2, (hi + 1) * H2)
                            ps2 = psp.tile([P, HID], F32, tag="ps")
                            nc.vector.tensor_copy(ps2[:, 0:H2], bhf_sb[1][:, hs])
                            for u in range(NU):
                                nc.tensor.matmul(
                                    ps2[:, 0:H2],
                                    hts[u][:, mc],
                                    whf_sb[1][u][:, hs],
                                    start=False, stop=(u == NU - 1),
                                    skip_group_check=True)
                            for q in range(2):
                                cs = slice(hi * H2 + q * (H2 // 2),
                                           hi * H2 + (q + 1) * (H2 // 2))
                                combine(cs, hi * 2 + q, ps2[:, 0:H2], hi * H2)

            # --- all layer-1 chunks first, then all layer-2 --------------
            for bc in range(nchunk):
                layer1(bc)
            for bc in range(nchunk):
                layer2(bc)

    nc.compile()
    return nc


def _prep_inputs(input, hx, ts, Wb, bb, W1, b1, W2, b2, Wa, ba, Wt, bt, bs=BS,
                 n_cores=N_CORES):
    import ml_dtypes
    f = np.float32
    h = np.float16
    f8 = ml_dtypes.float8_e4m3
    nchunk = bs // 512
    x = np.concatenate([np.asarray(input, f), np.asarray(hx, f)], axis=1)
    # pre-tiled [NK1, 2, 128, 512]: one contiguous block per DMA
    WbT = np.ascontiguousarray(
        np.asarray(Wb, f).T.astype(h)
        .reshape(NK1, P, 2, UNITS // 2).transpose(0, 2, 1, 3))
    WHF = np.stack([np.ascontiguousarray((1.7159 * np.asarray(W, f)).T.astype(h))
                    for W in (W1, W2)])                         # [2, 1024, 512]
    # fp8 DoubleRow pair layout: unit u = j*256 + s*128 + p -> [j, p, s, hid]
    WH8 = np.stack([
        np.ascontiguousarray(
            (K8 * 1.7159 * np.asarray(W, f)).T
            .reshape(NJ, 2, P, HID).transpose(0, 2, 1, 3).astype(f8))
        for W in (Wa, Wt)])                                     # [2, 4, 128, 2, 512]
    BBP = np.ascontiguousarray(
        (0.666 * np.asarray(bb, f)).reshape(NU, P).T)           # [128, 8]
    BHF = np.stack([np.ascontiguousarray(np.broadcast_to(np.asarray(b, f), (P, HID)))
                    for b in (b1, b2)])                         # [2, 128, 512]
    BH8 = np.stack([np.ascontiguousarray(np.broadcast_to(
        (K8 * np.asarray(b, f)).astype(f), (P, HID)))
        for b in (ba, bt)])                                     # [2, 128, 512]
    ts = np.asarray(ts, f).reshape(-1)
    xh = x.astype(h)

    in_maps = []
    for c in range(n_cores):
        lo, hi = c * bs, (c + 1) * bs
        in_maps.append({
            # pre-tiled [nchunk, NK1, 128, 512]: contiguous per-tile blocks
            "xt": np.ascontiguousarray(
                xh[lo:hi].T.reshape(NK1, P, nchunk, 512)
                .transpose(2, 0, 1, 3)),
            "wbt": WbT,
            "whf": WHF,
            "wh8": WH8,
            "bbp": BBP,
            "bhf": BHF,
            "bh8": BH8,
            "tsp": np.ascontiguousarray(ts[lo:hi].reshape(bs // P, P).T),
        })
    return in_maps


def kernel(input, hx, ts, Wb, bb, W1, b1, W2, b2, Wa, ba, Wt, bt):
    from concourse.bass_utils import run_bass_kernel_spmd

    if "nc" not in _cache:
        _cache["nc"] = build_nc()
    nc = _cache["nc"]

    in_maps = _prep_inputs(input, hx, ts, Wb, bb, W1, b1, W2, b2, Wa, ba, Wt, bt)
    trace = bool(int(os.environ.get("KERNEL_PROFILE", "0")))
    res = run_bass_kernel_spmd(nc, in_maps, list(range(N_CORES)), trace=trace)
    _cache["last_exec_time_ns"] = res.exec_time_ns
    _cache["last_results"] = res

    out = np.concatenate([res.results[c]["out"] for c in range(N_CORES)], axis=0)
    return out.astype(np.float32)


# revision 18
# speedup vs baseline: 1.0442x; 1.0442x over previous
"""CfC cell (dense MLP) Trainium2 Bass kernel.

Reference math (fp32):
    x  = concat([input, hx], axis=1)                  # [B, 768]
    h  = 1.7159 * tanh(0.666 * (x @ Wb.T + bb))       # [B, 1024]
    ff1 = tanh(h @ W1.T + b1)                         # [B, 512]
    ff2 = tanh(h @ W2.T + b2)
    t_a = h @ Wa.T + ba
    t_b = h @ Wt.T + bt
    t   = sigmoid(t_a * ts + t_b)
    out = ff1 * (1 - t) + t * ff2

Strategy: data-parallel over batch across 8 NeuronCores (2048 rows each).
Device layouts (contraction dim on partitions, fp16 matmul operands):
  - xT   [768, 2048]        x transposed
  - WbT  [768, 1024]        layer-1 stationary
  - WHF  [2, 1024, 512]     1.7159*W{1,2}.T, ff-head moving operands (fp16)
  - WH8  [2, 4, 128, 2, 512] K8*1.7159*W{a,t}.T as fp8e4m3 DoubleRow pairs
  - BBP  [128, 8]           0.666*bb (ACT bias for layer-1 tanh)
  - BHF/BH8 [2, 128, 512]   head biases broadcast across partitions (BH8 scaled K8)
  - TSP  [128, 16]          ts, column mi = batch subtile mi

Layer 1 emits hT [units, batch] twice from PSUM: fp16 tiles (ff heads) and
fp8 pair tiles (t heads).  The t-gate heads run as fp8 DoubleRow matmuls
(~1.44x PE rate); sigmoid's <=0.25 slope keeps the fp8 noise well inside the
accuracy budget.  All head biases are preloaded into PSUM by the (otherwise
idle) Pool engine so neither DVE nor ACT spends time on bias adds; the
accumulation groups then run start=False.  The final interpolation runs in
fp16 on DVE (2x rate).
"""

import os
import sys

import numpy as np

if "/opt/trn_rl_repo" not in sys.path:
    sys.path.insert(0, "/opt/trn_rl_repo")

B, IN, HID, UNITS = 16384, 256, 512, 1024
CAT = IN + HID  # 768
N_CORES = 8
BS = B // N_CORES  # 2048 per core
P = 128
NK1 = CAT // P    # 6 contraction tiles, layer 1
NU = UNITS // P   # 8 unit tiles
NJ = NU // 2      # 4 unit pair tiles (DoubleRow)
K8 = 1024.0       # fp8 weight pre-scale (sigmoid applies 1/K8)

_cache = {}


def build_nc(bs=BS, chunk=512):
    """Build the single-core Bass program (same program runs SPMD on 8 cores)."""
    from concourse import bacc, tile, mybir

    AF = mybir.ActivationFunctionType
    ALU = mybir.AluOpType
    PM = mybir.MatmulPerfMode
    F32 = mybir.dt.float32
    F16 = mybir.dt.float16
    F8 = mybir.dt.float8e4

    nchunk = bs // chunk
    nm = chunk // P  # batch subtiles per chunk
    NM = bs // P     # total batch subtiles

    nc = bacc.Bacc("TRN2", target_bir_lowering=False, debug=False,
                   num_devices=N_CORES)

    # xt/wbt are pre-tiled host-side so every startup DMA is one fully
    # contiguous block (the PE outruns scattered 1KB-row transfers)
    xt_d = nc.dram_tensor("xt", [nchunk, NK1, P, chunk], F16,
                          kind="ExternalInput").ap()
    wbt_d = nc.dram_tensor("wbt", [NK1, 2, P, UNITS // 2], F16,
                           kind="ExternalInput").ap()
    whf_d = nc.dram_tensor("whf", [2, UNITS, HID], F16, kind="ExternalInput").ap()
    wh8_d = nc.dram_tensor("wh8", [2, NJ, P, 2, HID], F8, kind="ExternalInput").ap()
    bbp_d = nc.dram_tensor("bbp", [P, NU], F32, kind="ExternalInput").ap()
    bhf_d = nc.dram_tensor("bhf", [2, P, HID], F32, kind="ExternalInput").ap()
    bh8_d = nc.dram_tensor("bh8", [2, P, HID], F32, kind="ExternalInput").ap()
    tsp_d = nc.dram_tensor("tsp", [P, NM], F32, kind="ExternalInput").ap()
    out_d = nc.dram_tensor("out", [bs, HID], F32, kind="ExternalOutput").ap()

    with tile.TileContext(nc) as tc:
        with (
            tc.tile_pool(name="const", bufs=1) as const,
            tc.tile_pool(name="xp", bufs=4) as xp,
            tc.tile_pool(name="hp", bufs=1) as hp,
            tc.tile_pool(name="tp", bufs=2) as tp,
            tc.tile_pool(name="op", bufs=3) as op,
            tc.tile_pool(name="psp", bufs=8, space="PSUM") as psp,
        ):
            # --- PE warmup: ramp the p-state while startup DMAs stream ----
            warm = const.tile([P, 512], F16, tag="warm")
            nc.vector.memset(warm[:], 0.0)
            for _ in range(3):
                wps = psp.tile([P, 512], F32, tag="ps")
                nc.tensor.matmul(wps[:], warm[:, 0:P], warm[:],
                                 start=True, stop=True)

            def load_x(bc):
                xts = []
                for c in range(NK1):
                    t = xp.tile([P, chunk], F16, tag=f"x{c}")
                    nc.sync.dma_start(t[:], xt_d[bc, c])
                    xts.append(t)
                return xts

            # first-chunk x tiles on the Sync DGE queue, layer-1 weights on
            # the Scalar DGE queue (idle until layer-1 ACT ~18us in): both
            # issue streams run in parallel so chunk 0 is fully resident
            # ~3.5us sooner than a single serialized queue
            HALF = UNITS // 2
            wb_sb = [[None, None] for _ in range(NK1)]
            xts0 = []
            for c in range(NK1):
                t = xp.tile([P, chunk], F16, tag=f"x{c}")
                # alternate the big x transfers across both queues so the
                # c-outer accumulation group never outruns the DMAs
                (nc.sync if c % 2 == 0 else nc.scalar).dma_start(
                    t[:], xt_d[0, c])
                xts0.append(t)
                t = const.tile([P, HALF], F16, tag=f"wbh{c}_0")
                (nc.scalar if c % 2 == 0 else nc.sync).dma_start(
                    t[:], wbt_d[c, 0])
                wb_sb[c][0] = t

            # small constants early (bb gates every layer-1 activation).
            # NOTE: the Scalar queue must carry only the 6 issues above —
            # anything more delays layer-1 ACTs (in-order engine) and stalls
            # PSUM recycling under the PE.
            bb_sb = const.tile([P, NU], F32, tag="bb")
            nc.sync.dma_start(bb_sb[:], bbp_d[:])

            for c in range(NK1):
                t = const.tile([P, HALF], F16, tag=f"wbh{c}_1")
                nc.sync.dma_start(t[:], wbt_d[c, 1])
                wb_sb[c][1] = t

            # all remaining x chunks next: layer-1 for every chunk runs
            # before any layer-2, so the head weights are needed only ~45us in
            xts_all = [xts0] + [load_x(bc) for bc in range(1, nchunk)]

            # t-head fp8 weights (needed first in each layer-2 m-tile)
            wh8_sb = [[None] * NJ for _ in range(2)]
            for k in range(2):
                for j in range(NJ):
                    t = const.tile([P, 2, HID], F8, tag=f"wh8_{k}_{j}")
                    nc.sync.dma_start(t[:], wh8_d[k, j])
                    wh8_sb[k][j] = t

            # ff-head fp16 weights
            whf_sb = [[None] * NU for _ in range(2)]
            for k in range(2):
                for u in range(NU):
                    t = const.tile([P, HID], F16, tag=f"whf_{k}_{u}")
                    nc.sync.dma_start(t[:], whf_d[k, u * P:(u + 1) * P, :])
                    whf_sb[k][u] = t

            # biases + ts (gate the first layer-2 psum preloads / stt)
            bh8_sb = [None, None]
            bhf_sb = [None, None]
            for k in range(2):
                t = const.tile([P, HID], F32, tag=f"bh8_{k}", name=f"bh8_{k}")
                nc.sync.dma_start(t[:], bh8_d[k])
                bh8_sb[k] = t
            for k in range(2):
                t = const.tile([P, HID], F32, tag=f"bhf_{k}", name=f"bhf_{k}")
                nc.sync.dma_start(t[:], bhf_d[k])
                bhf_sb[k] = t
            ts_sb = const.tile([P, NM], F32, tag="ts")
            nc.sync.dma_start(ts_sb[:], tsp_d[:])

            # persistent h storage: fp16 per u-tile, fp8 pairs per j-tile
            h16 = [[None] * NU for _ in range(nchunk)]
            h8 = [[None] * NJ for _ in range(nchunk)]
            for bc in range(nchunk):
                for u in range(NU):
                    h16[bc][u] = hp.tile([P, chunk], F16, tag=f"h16_{bc}_{u}",
                                         name=f"h16_{bc}_{u}")
                for j in range(NJ):
                    h8[bc][j] = hp.tile([P, 2, chunk], F8, tag=f"h8_{bc}_{j}",
                                        name=f"h8_{bc}_{j}")

            def layer1(bc):
                """hT[u] = tanh(0.666*(WbT.T @ xT) + 0.666*bb), fp16 + fp8 out.

                c-outer accumulation in two u-half-groups: the first matmul
                only needs xts[0] + wb half, so PE starts as soon as the
                first ~0.26 MB of DMA lands.
                """
                xts = xts_all[bc]
                for half in range(2):
                    pss = [psp.tile([P, chunk], F32, tag="ps", name=f"psl1_{j}")
                           for j in range(NU // 2)]
                    for c in range(NK1):
                        for j in range(NU // 2):
                            nc.tensor.matmul(
                                pss[j][:],
                                wb_sb[c][half][:, j * P:(j + 1) * P],
                                xts[c][:],
                                start=(c == 0), stop=(c == NK1 - 1))
                    for j in range(NU // 2):
                        u = half * (NU // 2) + j
                        nc.scalar.activation(h16[bc][u][:], pss[j][:], AF.Tanh,
                                             bias=bb_sb[:, u:u + 1], scale=0.666)
                        nc.scalar.activation(h8[bc][u // 2][:, u % 2, :],
                                             pss[j][:], AF.Tanh,
                                             bias=bb_sb[:, u:u + 1], scale=0.666)

            def layer2(bc):
                hts = h16[bc]
                h8s = h8[bc]
                for m in range(nm):
                    mi = bc * nm + m
                    last = (bc == nchunk - 1) and (m == nm - 1)
                    mc = slice(m * P, (m + 1) * P)

                    # t-path heads (fp8 DoubleRow), biases preloaded by Pool
                    def mm_t(k):
                        ps = psp.tile([P, HID], F32, tag="ps")
                        nc.vector.tensor_copy(ps[:], bh8_sb[k][:])
                        for j in range(NJ):
                            nc.tensor.matmul(
                                ps[:],
                                h8s[j][:, :, mc],
                                wh8_sb[k][j][:],
                                start=False, stop=(j == NJ - 1),
                                perf_mode=PM.DoubleRow,
                                skip_group_check=True)
                        return ps

                    def mm_ff(k):
                        ps = psp.tile([P, HID], F32, tag="ps")
                        nc.vector.tensor_copy(ps[:], bhf_sb[k][:])
                        for u in range(NU):
                            nc.tensor.matmul(
                                ps[:],
                                hts[u][:, mc],
                                whf_sb[k][u][:],
                                start=False, stop=(u == NU - 1),
                                skip_group_check=True)
                        return ps

                    # t-path first so the sigmoid chain overlaps the ff matmuls.
                    # DVE may read only one PSUM operand per op: stage pb in
                    # SBUF via ACT (overlaps pa's matmuls).
                    pb = mm_t(1)
                    ub = tp.tile([P, HID], F32, tag="ub")
                    nc.scalar.copy(ub[:], pb[:])
                    pa = mm_t(0)
                    w = tp.tile([P, HID], F32, tag="w")
                    nc.vector.scalar_tensor_tensor(
                        w[:], pa[:], ts_sb[:, mi:mi + 1], ub[:],
                        op0=ALU.mult, op1=ALU.add)
                    tt = tp.tile([P, HID], F16, tag="tt")
                    nc.scalar.activation(tt[:], w[:], AF.Sigmoid, scale=1.0 / K8)

                    p1 = mm_ff(0)
                    f1 = tp.tile([P, HID], F16, tag="f1")
                    nc.scalar.activation(f1[:], p1[:], AF.Tanh)

                    f2 = tp.tile([P, HID], F16, tag="f2")
                    o = op.tile([P, HID], F32, tag="o")

                    def combine(cs, qi, p2t, lo):
                        """o[:, cs] = f1 + tt*(f2 - f1); p2t covers cols lo:."""
                        ls = slice(cs.start - lo, cs.stop - lo)
                        d = tp.tile([P, HID], F16, tag="d")
                        nc.scalar.activation(f2[:, cs], p2t[:, ls], AF.Tanh)
                        nc.vector.tensor_sub(d[:, cs], f2[:, cs], f1[:, cs])
                        nc.vector.tensor_mul(d[:, cs], d[:, cs], tt[:, cs])
                        nc.vector.tensor_add(o[:, cs], d[:, cs], f1[:, cs])
                        # alternate DGE queues so the tail quarters' DMA
                        # issues (~600ns each) overlap instead of serializing
                        eng = nc.scalar if (last and qi % 2) else nc.sync
                        eng.dma_start(out_d[mi * P:(mi + 1) * P, cs], o[:, cs])

                    if not last:
                        p2 = mm_ff(1)
                        combine(slice(0, HID), 0, p2, 0)
                    else:
                        # last tile: ff2 in two 256-col half-groups so the
                        # first half's chain overlaps the second half's
                        # matmuls, and quartered chains pipeline ACT/DVE/DMA
                        H2 = HID // 2
                        for hi in range(2):
                            hs = slice(hi * H2, (hi + 1) * H2)
                            ps2 = psp.tile([P, HID], F32, tag="ps")
                            nc.vector.tensor_copy(ps2[:, 0:H2], bhf_sb[1][:, hs])
                            for u in range(NU):
                                nc.tensor.matmul(
                                    ps2[:, 0:H2],
                                    hts[u][:, mc],
                                    whf_sb[1][u][:, hs],
                                    start=False, stop=(u == NU - 1),
                                    skip_group_check=True)
                            for q in range(2):
                                cs = slice(hi * H2 + q * (H2 // 2),
                                           hi * H2 + (q + 1) * (H2 // 2))
                                combine(cs, hi * 2 + q, ps2[:, 0:H2], hi * H2)

            # --- all layer-1 chunks first, then all layer-2 --------------
            for bc in range(nchunk):
                layer1(bc)
            for bc in range(nchunk):
                layer2(bc)

    nc.compile()
    return nc


def _prep_inputs(input, hx, ts, Wb, bb, W1, b1, W2, b2, Wa, ba, Wt, bt, bs=BS,
                 n_cores=N_CORES):
    import ml_dtypes
    f = np.float32
    h = np.float16
    f8 = ml_dtypes.float8_e4m3
    nchunk = bs // 512
    x = np.concatenate([np.asarray(input, f), np.asarray(hx, f)], axis=1)
    # pre-tiled [NK1, 2, 128, 512]: one contiguous block per DMA
    WbT = np.ascontiguousarray(
        np.asarray(Wb, f).T.astype(h)
        .reshape(NK1, P, 2, UNITS // 2).transpose(0, 2, 1, 3))
    WHF = np.stack([np.ascontiguousarray((1.7159 * np.asarray(W, f)).T.astype(h))
                    for W in (W1, W2)])                         # [2, 1024, 512]
    # fp8 DoubleRow pair layout: unit u = j*256 + s*128 + p -> [j, p, s, hid]
    WH8 = np.stack([
        np.ascontiguousarray(
            (K8 * 1.7159 * np.asarray(W, f)).T
            .reshape(NJ, 2, P, HID).transpose(0, 2, 1, 3).astype(f8))
        for W in (Wa, Wt)])                                     # [2, 4, 128, 2, 512]
    BBP = np.ascontiguousarray(
        (0.666 * np.asarray(bb, f)).reshape(NU, P).T)           # [128, 8]
    BHF = np.stack([np.ascontiguousarray(np.broadcast_to(np.asarray(b, f), (P, HID)))
                    for b in (b1, b2)])                         # [2, 128, 512]
    BH8 = np.stack([np.ascontiguousarray(np.broadcast_to(
        (K8 * np.asarray(b, f)).astype(f), (P, HID)))
        for b in (ba, bt)])                                     # [2, 128, 512]
    ts = np.asarray(ts, f).reshape(-1)
    xh = x.astype(h)

    in_maps = []
    for c in range(n_cores):
        lo, hi = c * bs, (c + 1) * bs
        in_maps.append({
            # pre-tiled [nchunk, NK1, 128, 512]: contiguous per-tile blocks
            "xt": np.ascontiguousarray(
                xh[lo:hi].T.reshape(NK1, P, nchunk, 512)
                .transpose(2, 0, 1, 3)),
            "wbt": WbT,
            "whf": WHF,
            "wh8": WH8,
            "bbp": BBP,
            "bhf": BHF,
            "bh8": BH8,
            "tsp": np.ascontiguousarray(ts[lo:hi].reshape(bs // P, P).T),
        })
    return in_maps


def kernel(input, hx, ts, Wb, bb, W1, b1, W2, b2, Wa, ba, Wt, bt):
    from concourse.bass_utils import run_bass_kernel_spmd

    if "nc" not in _cache:
        _cache["nc"] = build_nc()
    nc = _cache["nc"]

    in_maps = _prep_inputs(input, hx, ts, Wb, bb, W1, b1, W2, b2, Wa, ba, Wt, bt)
    trace = bool(int(os.environ.get("KERNEL_PROFILE", "0")))
    res = run_bass_kernel_spmd(nc, in_maps, list(range(N_CORES)), trace=trace)
    _cache["last_exec_time_ns"] = res.exec_time_ns
    _cache["last_results"] = res

    out = np.concatenate([res.results[c]["out"] for c in range(N_CORES)], axis=0)
    return out.astype(np.float32)


# revision 23
# speedup vs baseline: 1.0505x; 1.0061x over previous
"""CfC cell (dense MLP) Trainium2 Bass kernel.

Reference math (fp32):
    x  = concat([input, hx], axis=1)                  # [B, 768]
    h  = 1.7159 * tanh(0.666 * (x @ Wb.T + bb))       # [B, 1024]
    ff1 = tanh(h @ W1.T + b1)                         # [B, 512]
    ff2 = tanh(h @ W2.T + b2)
    t_a = h @ Wa.T + ba
    t_b = h @ Wt.T + bt
    t   = sigmoid(t_a * ts + t_b)
    out = ff1 * (1 - t) + t * ff2

Strategy: data-parallel over batch across 8 NeuronCores (2048 rows each).
Device layouts (contraction dim on partitions, fp16 matmul operands):
  - xT   [768, 2048]        x transposed
  - WbT  [768, 1024]        layer-1 stationary
  - WHF  [2, 1024, 512]     1.7159*W{1,2}.T, ff-head moving operands (fp16)
  - WH8  [2, 4, 128, 2, 512] K8*1.7159*W{a,t}.T as fp8e4m3 DoubleRow pairs
  - BBP  [128, 8]           0.666*bb (ACT bias for layer-1 tanh)
  - BHF/BH8 [2, 128, 512]   head biases broadcast across partitions (BH8 scaled K8)
  - TSP  [128, 16]          ts, column mi = batch subtile mi

Layer 1 emits hT [units, batch] twice from PSUM: fp16 tiles (ff heads) and
fp8 pair tiles (t heads).  The t-gate heads run as fp8 DoubleRow matmuls
(~1.44x PE rate); sigmoid's <=0.25 slope keeps the fp8 noise well inside the
accuracy budget.  All head biases are preloaded into PSUM by the (otherwise
idle) Pool engine so neither DVE nor ACT spends time on bias adds; the
accumulation groups then run start=False.  The final interpolation runs in
fp16 on DVE (2x rate).
"""

import os
import sys

import numpy as np

if "/opt/trn_rl_repo" not in sys.path:
    sys.path.insert(0, "/opt/trn_rl_repo")

B, IN, HID, UNITS = 16384, 256, 512, 1024
CAT = IN + HID  # 768
N_CORES = 8
BS = B // N_CORES  # 2048 per core
P = 128
NK1 = CAT // P    # 6 contraction tiles, layer 1
NU = UNITS // P   # 8 unit tiles
NJ = NU // 2      # 4 unit pair tiles (DoubleRow)
K8 = 1024.0       # fp8 weight pre-scale (sigmoid applies 1/K8)

_cache = {}


def build_nc(bs=BS, chunk=512):
    """Build the single-core Bass program (same program runs SPMD on 8 cores)."""
    from concourse import bacc, tile, mybir

    AF = mybir.ActivationFunctionType
    ALU = mybir.AluOpType
    PM = mybir.MatmulPerfMode
    F32 = mybir.dt.float32
    F16 = mybir.dt.float16
    F8 = mybir.dt.float8e4

    nchunk = bs // chunk
    nm = chunk // P  # batch subtiles per chunk
    NM = bs // P     # total batch subtiles

    nc = bacc.Bacc("TRN2", target_bir_lowering=False, debug=False,
                   num_devices=N_CORES)

    # xt/wbt are pre-tiled host-side so every startup DMA is one fully
    # contiguous block (the PE outruns scattered 1KB-row transfers)
    xt_d = nc.dram_tensor("xt", [nchunk, NK1, P, chunk], F16,
                          kind="ExternalInput").ap()
    wbt_d = nc.dram_tensor("wbt", [NK1, 2, P, UNITS // 2], F16,
                           kind="ExternalInput").ap()
    whf_d = nc.dram_tensor("whf", [2, UNITS, HID], F16, kind="ExternalInput").ap()
    wh8_d = nc.dram_tensor("wh8", [2, NJ, P, 2, HID], F8, kind="ExternalInput").ap()
    bbp_d = nc.dram_tensor("bbp", [P, NU], F32, kind="ExternalInput").ap()
    bhf_d = nc.dram_tensor("bhf", [2, P, HID], F32, kind="ExternalInput").ap()
    bh8_d = nc.dram_tensor("bh8", [2, P, HID], F32, kind="ExternalInput").ap()
    tsp_d = nc.dram_tensor("tsp", [P, NM], F32, kind="ExternalInput").ap()
    out_d = nc.dram_tensor("out", [bs, HID], F16, kind="ExternalOutput").ap()

    with tile.TileContext(nc) as tc:
        with (
            tc.tile_pool(name="const", bufs=1) as const,
            tc.tile_pool(name="xp", bufs=4) as xp,
            tc.tile_pool(name="hp", bufs=1) as hp,
            tc.tile_pool(name="tp", bufs=2) as tp,
            tc.tile_pool(name="op", bufs=3) as op,
            tc.tile_pool(name="psp", bufs=8, space="PSUM") as psp,
        ):
            # --- PE warmup: ramp the p-state while startup DMAs stream ----
            warm = const.tile([P, 512], F16, tag="warm")
            nc.gpsimd.memset(warm[:], 0.0)
            for _ in range(2):
                wps = psp.tile([P, 512], F32, tag="ps")
                nc.tensor.matmul(wps[:], warm[:, 0:P], warm[:],
                                 start=True, stop=True)

            def load_x(bc):
                xts = []
                for c in range(NK1):
                    t = xp.tile([P, chunk], F16, tag=f"x{c}")
                    nc.sync.dma_start(t[:], xt_d[bc, c])
                    xts.append(t)
                return xts

            # first-chunk x tiles on the Sync DGE queue, layer-1 weights on
            # the Scalar DGE queue (idle until layer-1 ACT ~18us in): both
            # issue streams run in parallel so chunk 0 is fully resident
            # ~3.5us sooner than a single serialized queue
            HALF = UNITS // 2
            wb_sb = [[None, None] for _ in range(NK1)]
            xts0 = []
            for c in range(NK1):
                t = xp.tile([P, chunk], F16, tag=f"x{c}")
                if c == 0:
                    # first matmul's gate: split x0/wbh0 into half transfers
                    # on both queues so they land ~1us sooner
                    nc.sync.dma_start(t[:, 0:chunk // 2],
                                      xt_d[0, 0, :, 0:chunk // 2])
                    nc.scalar.dma_start(t[:, chunk // 2:chunk],
                                        xt_d[0, 0, :, chunk // 2:chunk])
                else:
                    # alternate the big x transfers across both queues so the
                    # c-outer accumulation group never outruns the DMAs
                    (nc.sync if c % 2 == 0 else nc.scalar).dma_start(
                        t[:], xt_d[0, c])
                xts0.append(t)
                t = const.tile([P, HALF], F16, tag=f"wbh{c}_0")
                if c == 0:
                    nc.scalar.dma_start(t[:, 0:HALF // 2],
                                        wbt_d[0, 0, :, 0:HALF // 2])
                    nc.sync.dma_start(t[:, HALF // 2:HALF],
                                      wbt_d[0, 0, :, HALF // 2:HALF])
                else:
                    (nc.scalar if c % 2 == 0 else nc.sync).dma_start(
                        t[:], wbt_d[c, 0])
                wb_sb[c][0] = t

            # small constants early (bb gates every layer-1 activation).
            # NOTE: the Scalar queue must carry only the 6 issues above —
            # anything more delays layer-1 ACTs (in-order engine) and stalls
            # PSUM recycling under the PE.
            bb_sb = const.tile([P, NU], F32, tag="bb")
            nc.sync.dma_start(bb_sb[:], bbp_d[:])

            for c in range(NK1):
                t = const.tile([P, HALF], F16, tag=f"wbh{c}_1")
                nc.sync.dma_start(t[:], wbt_d[c, 1])
                wb_sb[c][1] = t

            # all remaining x chunks next: layer-1 for every chunk runs
            # before any layer-2, so the head weights are needed only ~45us in
            xts_all = [xts0] + [load_x(bc) for bc in range(1, nchunk)]

            # t-head fp8 weights (needed first in each layer-2 m-tile)
            wh8_sb = [[None] * NJ for _ in range(2)]
            for k in range(2):
                for j in range(NJ):
                    t = const.tile([P, 2, HID], F8, tag=f"wh8_{k}_{j}")
                    nc.sync.dma_start(t[:], wh8_d[k, j])
                    wh8_sb[k][j] = t

            # ff-head fp16 weights
            whf_sb = [[None] * NU for _ in range(2)]
            for k in range(2):
                for u in range(NU):
                    t = const.tile([P, HID], F16, tag=f"whf_{k}_{u}")
                    nc.sync.dma_start(t[:], whf_d[k, u * P:(u + 1) * P, :])
                    whf_sb[k][u] = t

            # biases + ts (gate the first layer-2 psum preloads / stt)
            bh8_sb = [None, None]
            bhf_sb = [None, None]
            for k in range(2):
                t = const.tile([P, HID], F32, tag=f"bh8_{k}", name=f"bh8_{k}")
                nc.sync.dma_start(t[:], bh8_d[k])
                bh8_sb[k] = t
            for k in range(2):
                t = const.tile([P, HID], F32, tag=f"bhf_{k}", name=f"bhf_{k}")
                nc.sync.dma_start(t[:], bhf_d[k])
                bhf_sb[k] = t
            ts_sb = const.tile([P, NM], F32, tag="ts")
            nc.sync.dma_start(ts_sb[:], tsp_d[:])

            # persistent h storage: fp16 per u-tile, fp8 pairs per j-tile
            h16 = [[None] * NU for _ in range(nchunk)]
            h8 = [[None] * NJ for _ in range(nchunk)]
            for bc in range(nchunk):
                for u in range(NU):
                    h16[bc][u] = hp.tile([P, chunk], F16, tag=f"h16_{bc}_{u}",
                                         name=f"h16_{bc}_{u}")
                for j in range(NJ):
                    h8[bc][j] = hp.tile([P, 2, chunk], F8, tag=f"h8_{bc}_{j}",
                                        name=f"h8_{bc}_{j}")

            def layer1(bc):
                """hT[u] = tanh(0.666*(WbT.T @ xT) + 0.666*bb), fp16 + fp8 out.

                c-outer accumulation in two u-half-groups: the first matmul
                only needs xts[0] + wb half, so PE starts as soon as the
                first ~0.26 MB of DMA lands.
                """
                xts = xts_all[bc]
                for half in range(2):
                    pss = [psp.tile([P, chunk], F32, tag="ps", name=f"psl1_{j}")
                           for j in range(NU // 2)]
                    for c in range(NK1):
                        for j in range(NU // 2):
                            nc.tensor.matmul(
                                pss[j][:],
                                wb_sb[c][half][:, j * P:(j + 1) * P],
                                xts[c][:],
                                start=(c == 0), stop=(c == NK1 - 1))
                    for j in range(NU // 2):
                        u = half * (NU // 2) + j
                        nc.scalar.activation(h16[bc][u][:], pss[j][:], AF.Tanh,
                                             bias=bb_sb[:, u:u + 1], scale=0.666)
                        nc.scalar.activation(h8[bc][u // 2][:, u % 2, :],
                                             pss[j][:], AF.Tanh,
                                             bias=bb_sb[:, u:u + 1], scale=0.666)

            def layer2(bc):
                hts = h16[bc]
                h8s = h8[bc]
                for m in range(nm):
                    mi = bc * nm + m
                    last = (bc == nchunk - 1) and (m == nm - 1)
                    mc = slice(m * P, (m + 1) * P)

                    # t-path heads (fp8 DoubleRow), biases preloaded by Pool
                    def mm_t(k):
                        ps = psp.tile([P, HID], F32, tag="ps")
                        nc.vector.tensor_copy(ps[:], bh8_sb[k][:])
                        for j in range(NJ):
                            nc.tensor.matmul(
                                ps[:],
                                h8s[j][:, :, mc],
                                wh8_sb[k][j][:],
                                start=False, stop=(j == NJ - 1),
                                perf_mode=PM.DoubleRow,
                                skip_group_check=True)
                        return ps

                    def mm_ff(k):
                        ps = psp.tile([P, HID], F32, tag="ps")
                        nc.vector.tensor_copy(ps[:], bhf_sb[k][:])
                        for u in range(NU):
                            nc.tensor.matmul(
                                ps[:],
                                hts[u][:, mc],
                                whf_sb[k][u][:],
                                start=False, stop=(u == NU - 1),
                                skip_group_check=True)
                        return ps

                    # t-path first so the sigmoid chain overlaps the ff matmuls.
                    # DVE may read only one PSUM operand per op: stage pb in
                    # SBUF via ACT (overlaps pa's matmuls).
                    pb = mm_t(1)
                    ub = tp.tile([P, HID], F32, tag="ub")
                    nc.scalar.copy(ub[:], pb[:])
                    pa = mm_t(0)
                    w = tp.tile([P, HID], F32, tag="w")
                    nc.vector.scalar_tensor_tensor(
                        w[:], pa[:], ts_sb[:, mi:mi + 1], ub[:],
                        op0=ALU.mult, op1=ALU.add)
                    tt = tp.tile([P, HID], F16, tag="tt")
                    nc.scalar.activation(tt[:], w[:], AF.Sigmoid, scale=1.0 / K8)

                    p1 = mm_ff(0)
                    f1 = tp.tile([P, HID], F16, tag="f1")
                    nc.scalar.activation(f1[:], p1[:], AF.Tanh)

                    f2 = tp.tile([P, HID], F16, tag="f2")
                    o = op.tile([P, HID], F16, tag="o")

                    def combine(cs, qi, p2t, lo):
                        """o[:, cs] = f1 + tt*(f2 - f1); p2t covers cols lo:."""
                        ls = slice(cs.start - lo, cs.stop - lo)
                        d = tp.tile([P, HID], F16, tag="d")
                        nc.scalar.activation(f2[:, cs], p2t[:, ls], AF.Tanh)
                        nc.vector.tensor_sub(d[:, cs], f2[:, cs], f1[:, cs])
                        nc.vector.tensor_mul(d[:, cs], d[:, cs], tt[:, cs])
                        nc.vector.tensor_add(o[:, cs], d[:, cs], f1[:, cs])
                        # alternate DGE queues so the tail quarters' DMA
                        # issues (~600ns each) overlap instead of serializing
                        eng = nc.scalar if (last and qi % 2) else nc.sync
                        eng.dma_start(out_d[mi * P:(mi + 1) * P, cs], o[:, cs])

                    if not last:
                        p2 = mm_ff(1)
                        combine(slice(0, HID), 0, p2, 0)
                    else:
                        # last tile: ff2 in four 128-col groups so each
                        # group's ACT/DVE/DMA chain starts the moment its
                        # matmuls retire — minimal work after the final matmul
                        H4 = HID // 4
                        for qi in range(4):
                            hs = slice(qi * H4, (qi + 1) * H4)
                            ps2 = psp.tile([P, HID], F32, tag="ps")
                            nc.vector.tensor_copy(ps2[:, 0:H4], bhf_sb[1][:, hs])
                            for u in range(NU):
                                nc.tensor.matmul(
                                    ps2[:, 0:H4],
                                    hts[u][:, mc],
                                    whf_sb[1][u][:, hs],
                                    start=False, stop=(u == NU - 1),
                                    skip_group_check=True)
                            combine(hs, qi, ps2[:, 0:H4], qi * H4)

            # --- all layer-1 chunks first, then all layer-2 --------------
            for bc in range(nchunk):
                layer1(bc)
            for bc in range(nchunk):
                layer2(bc)

    nc.compile()
    return nc


def _prep_inputs(input, hx, ts, Wb, bb, W1, b1, W2, b2, Wa, ba, Wt, bt, bs=BS,
                 n_cores=N_CORES):
    import ml_dtypes
    f = np.float32
    h = np.float16
    f8 = ml_dtypes.float8_e4m3
    nchunk = bs // 512
    x = np.concatenate([np.asarray(input, f), np.asarray(hx, f)], axis=1)
    # pre-tiled [NK1, 2, 128, 512]: one contiguous block per DMA
    WbT = np.ascontiguousarray(
        np.asarray(Wb, f).T.astype(h)
        .reshape(NK1, P, 2, UNITS // 2).transpose(0, 2, 1, 3))
    WHF = np.stack([np.ascontiguousarray((1.7159 * np.asarray(W, f)).T.astype(h))
                    for W in (W1, W2)])                         # [2, 1024, 512]
    # fp8 DoubleRow pair layout: unit u = j*256 + s*128 + p -> [j, p, s, hid]
    WH8 = np.stack([
        np.ascontiguousarray(
            (K8 * 1.7159 * np.asarray(W, f)).T
            .reshape(NJ, 2, P, HID).transpose(0, 2, 1, 3).astype(f8))
        for W in (Wa, Wt)])                                     # [2, 4, 128, 2, 512]
    BBP = np.ascontiguousarray(
        (0.666 * np.asarray(bb, f)).reshape(NU, P).T)           # [128, 8]
    BHF = np.stack([np.ascontiguousarray(np.broadcast_to(np.asarray(b, f), (P, HID)))
                    for b in (b1, b2)])                         # [2, 128, 512]
    BH8 = np.stack([np.ascontiguousarray(np.broadcast_to(
        (K8 * np.asarray(b, f)).astype(f), (P, HID)))
        for b in (ba, bt)])                                     # [2, 128, 512]
    ts = np.asarray(ts, f).reshape(-1)
    xh = x.astype(h)

    in_maps = []
    for c in range(n_cores):
        lo, hi = c * bs, (c + 1) * bs
        in_maps.append({
            # pre-tiled [nchunk, NK1, 128, 512]: contiguous per-tile blocks
            "xt": np.ascontiguousarray(
                xh[lo:hi].T.reshape(NK1, P, nchunk, 512)
                .transpose(2, 0, 1, 3)),
            "wbt": WbT,
            "whf": WHF,
            "wh8": WH8,
            "bbp": BBP,
            "bhf": BHF,
            "bh8": BH8,
            "tsp": np.ascontiguousarray(ts[lo:hi].reshape(bs // P, P).T),
        })
    return in_maps


def kernel(input, hx, ts, Wb, bb, W1, b1, W2, b2, Wa, ba, Wt, bt):
    from concourse.bass_utils import run_bass_kernel_spmd

    if "nc" not in _cache:
        _cache["nc"] = build_nc()
    nc = _cache["nc"]

    in_maps = _prep_inputs(input, hx, ts, Wb, bb, W1, b1, W2, b2, Wa, ba, Wt, bt)
    trace = bool(int(os.environ.get("KERNEL_PROFILE", "0")))
    res = run_bass_kernel_spmd(nc, in_maps, list(range(N_CORES)), trace=trace)
    _cache["last_exec_time_ns"] = res.exec_time_ns
    _cache["last_results"] = res

    # device emits fp16 (halves the output DMA); widen host-side
    out = np.concatenate([res.results[c]["out"] for c in range(N_CORES)], axis=0)
    return out.astype(np.float32)
